# revision 19
# baseline (speedup 1.0000x reference)
"""
Trainium2 Bass kernel for nn_NodeEquiModel (gnn_message_passing).

Computation (reference, jax):
    fn = equi_rep(f_nodes)            # [N, 2, 45]  (45-of-81 selection per 9x9 block)
    fe = equi_rep(f_edges)            # [E, 2, 45]
    fn = fn[edge_index[0]]            # gather -> [E, 2, 45]
    tp[e,c,k] = sum_ij fn[e,c,i] fe[e,c,j] W_tp[i,j,k] / 45
    out = (tp @ W_fc1)/sqrt(32) @ W_fc2 / sqrt(64)    # [E, 2, 45]

Device strategy (8 cores, edges sharded, 50048 edges/core):
  64-edge tiles with channels packed into partitions: rows 0-63 = ch0,
  rows 64-127 = ch1 of the same 64 edges.  Host precomputes (all fp16):
    FT  [110, T*64]  voigt(fe)^T        (ch0 rows 0-45, ch1 rows 64-109)
    FN  [T*128, 48]  voigt(f_nodes)[row] gathered rows, (c,e)-packed
    W2  [110, 1472]  W'[j,(k,i46)] = W_tp[i,j,k]/45 at rows 0-45 and 64-109
    Mfc [ 4*32, 45]  (W_fc1@W_fc2)/sqrt(32*64) replicated at partitions 0,32,64,96
  Per tile:
    PE   pass-1 (2-way array tiling, ch0/ch1 concurrent):
         u[0:64,(k,i)]  = FT_ch0^T @ W2   (3 chunks <= 512)
         u[64:128,...]  = FT_ch1^T @ W2   (quadrant (64,64))
    ACT  evac u -> u16 fp16 (k < KA); DVE fused-mult the rest from PSUM
    DVE/Pool  prod = u16 * fn (fp16 @2x; Pool takes k < KP)
    DVE  fold i-halves (pair-merged), tensor_reduce over 23 -> tp fp16
    PE   (per tile pair) transpose tp2 -> tpT; FC 2-way tiled -> oT fp32
    ACT  evacs; DMA out fp16 [45+45 rows, 128].
  Host: inverse layout -> [E, 2, 45] fp32.
"""

import math

import numpy as np

import concourse.bass as bass
import concourse.mybir as mybir
import concourse.tile as tile
from concourse.bass_utils import run_bass_kernel_spmd

# ---------------------------------------------------------------- constants
N_NODES = 100000
N_EDGES = 400000
MB = 9
RAW = MB * MB          # 81
REP = 45
IV = 46                # padded i dim (45 + 1)
OUT_K = 32
KJ = OUT_K * IV        # 1472
N_CORES = 8

TILE_E = 64            # edges per tile (x2 channels = 128 partitions)
E_PER_CORE = N_EDGES // N_CORES            # 50000
N_TILES = math.ceil(E_PER_CORE / TILE_E)   # 782
E_PAD = N_TILES * TILE_E                   # 50048

KA = 32                # k < KA evac'd by ACT (all of U)
KP = 22                # k < KP: fp16 mult on Pool; KP<=k: fp16 mult on DVE
F16 = mybir.dt.float16
F32 = mybir.dt.float32


def _voigt_sel():
    """45 flat indices into the 81-element 9x9 block, in generate_equi_rep order."""
    idx = [0]
    idx += [9 * i + i for i in range(1, 4)]
    iu, ju = np.triu_indices(3, 1)
    idx += [9 * (i + 1) + (j + 1) for i, j in zip(iu, ju)]
    idx += [9 * i + i for i in range(4, 9)]
    iu, ju = np.triu_indices(5, 1)
    idx += [9 * (i + 4) + (j + 4) for i, j in zip(iu, ju)]
    idx += [j for j in range(1, 4)]
    idx += [j for j in range(4, 9)]
    idx += [9 * i + j for i in range(1, 4) for j in range(4, 9)]
    assert len(idx) == 45 and len(set(idx)) == 45
    return np.array(idx, dtype=np.int64)


def _split_excess_waits(nc):
    """PE matmuls and DMA pseudo-instructions can carry at most ONE sync wait
    on TRN2 (walrus codegen: 'Too many sync wait commands'). Move excess waits
    onto a standalone NoOp on the same engine stream right before the
    instruction."""
    import bass_rust

    f = nc.m.functions[0]
    for b in f.blocks:
        il = b.instructions
        k = 0
        while k < len(il):
            inst = il[k]
            si = inst.sync_info
            if si is not None and len(si.on_wait) > 1:
                moved = list(si.on_wait[:-1])
                kept = [si.on_wait[-1]]
                for w in moved:
                    nop = bass_rust.InstNoOp(name=f"I-wsplit-{nc.next_id()}", ins=[], outs=[])
                    nop.engine = inst.engine
                    nop.sync_info = bass_rust.SyncInfo(on_wait=[w], on_update=[])
                    il.insert(k, nop)
                    k += 1
                inst.sync_info = bass_rust.SyncInfo(on_wait=kept,
                                                    on_update=list(si.on_update))
            k += 1


def _build_bass():
    nc = bass.Bass()

    in_d = nc.declare_dram_parameter("inp", [128, N_TILES // 2, 224], F16, isOutput=False)
    w2_d = nc.declare_dram_parameter("w2", [110, KJ], F16, isOutput=False)
    mfc_d = nc.declare_dram_parameter("mfc", [128, REP], F16, isOutput=False)
    ident_d = nc.declare_dram_parameter("ident", [128, 128], F16, isOutput=False)
    out_d = nc.declare_dram_parameter("out_shard", [128, N_TILES // 2, 128], F16, isOutput=True)

    NK_CHUNKS = [(0, 512), (512, 1024), (1024, KJ)]
    A = KA * IV            # 920 ACT-evac'd columns
    lp = None

    with tile.TileContext(nc) as tc:
        with (
            tc.tile_pool(name="consts", bufs=1) as consts,
            tc.tile_pool(name="io", bufs=4) as io,
            tc.tile_pool(name="mid", bufs=3) as mid,
            tc.tile_pool(name="tps", bufs=3) as tps,
            tc.tile_pool(name="psu", bufs=2, space="PSUM") as psu,
            tc.tile_pool(name="psfc", bufs=1, space="PSUM") as psfc,
        ):
            w2 = consts.tile([110, KJ], F16, tag="w2")
            nc.sync.dma_start(out=w2[:], in_=w2_d[:])
            mfc = consts.tile([128, REP], F16, tag="mfc")
            nc.sync.dma_start(out=mfc[:], in_=mfc_d[:])
            ident = consts.tile([128, 128], F16, tag="id")
            nc.sync.dma_start(out=ident[:], in_=ident_d[:])

            # Preamble warm-up: absorb const-DMA deps into each engine's clock
            # before the loop (PE matmuls carry only one HW sync wait).
            warm = psu.tile([128, KJ], F32, tag="u")
            nc.tensor.matmul(warm[0:64, 0:128], lhsT=w2[0:46, 0:64],
                             rhs=w2[0:46, 0:128], start=True, stop=True,
                             tile_position=(0, 0))
            nc.tensor.matmul(warm[64:128, 0:128], lhsT=w2[64:110, 0:64],
                             rhs=w2[64:110, 0:128], start=True, stop=True,
                             tile_position=(64, 64))
            warmT = psfc.tile([64, 128], F16, tag="tpT_ps")
            nc.tensor.transpose(warmT[:], ident[:, 0:64], ident[:])
            warm2 = psfc.tile([128, 128], F32, tag="o")
            nc.tensor.matmul(warm2[0:45, 0:64], lhsT=mfc[0:32, 0:45],
                             rhs=ident[0:32, 0:64], start=True, stop=True,
                             tile_position=(0, 0))
            nc.tensor.matmul(warm2[64:109, 64:128], lhsT=mfc[32:64, 0:45],
                             rhs=ident[32:64, 0:64], start=True, stop=True,
                             tile_position=(32, 64))

            lp = nc.allow_low_precision("fp16 pipeline; fp32 accumulation on PE")
            lp.__enter__()

            for tp2 in range(N_TILES // 2):
                # ---- pair inputs: one packed DMA
                inp = io.tile([128, 224], F16, tag="inp")
                nc.sync.dma_start(out=inp[:], in_=in_d[:, tp2, :])
                ft = inp[:, 0:128]
                fnv = inp[:, 128:224]

                u16 = mid.tile([128, 2 * KJ], F16, tag="u16")
                for half in range(2):
                    # ---- pass-1: 2-way PE array tiling, channels concurrent
                    u_ps = psu.tile([128, KJ], F32, tag="u")
                    e0 = half * TILE_E
                    for (a, b) in NK_CHUNKS:
                        nc.tensor.matmul(u_ps[0:64, a:b],
                                         lhsT=ft[0:46, e0:e0 + TILE_E],
                                         rhs=w2[0:46, a:b], start=True, stop=True,
                                         tile_position=(0, 0))
                        nc.tensor.matmul(u_ps[64:128, a:b],
                                         lhsT=ft[64:110, e0:e0 + TILE_E],
                                         rhs=w2[64:110, a:b], start=True, stop=True,
                                         tile_position=(64, 64))
                    # ---- ACT evac into pair buffer half
                    nc.scalar.copy(out=u16[:, half * KJ:(half + 1) * KJ], in_=u_ps[:])

                # ---- multiplies -> prod fp16 [128, (2t, k32, i46)], one op per engine
                prod = mid.tile([128, 2 * KJ], F16, tag="prod")
                p4 = prod[:].rearrange("p (t k i) -> p t k i", t=2, k=OUT_K)
                u16_4 = u16[:].rearrange("p (t k i) -> p t k i", t=2, k=OUT_K)
                fn_b = fnv.rearrange("p (t a i) -> p t a i", t=2, a=1)[:, :, :, 0:IV]
                # Pool: k < KP, two ops (Q7 throughput degrades on large single ops)
                nc.gpsimd.tensor_tensor(
                    out=p4[:, :, 0:KP // 2, :], in0=u16_4[:, :, 0:KP // 2, :],
                    in1=fn_b.to_broadcast([128, 2, KP // 2, IV]),
                    op=mybir.AluOpType.mult)
                nc.gpsimd.tensor_tensor(
                    out=p4[:, :, KP // 2:KP, :], in0=u16_4[:, :, KP // 2:KP, :],
                    in1=fn_b.to_broadcast([128, 2, KP - KP // 2, IV]),
                    op=mybir.AluOpType.mult)
                # DVE: KP <= k
                nc.vector.tensor_tensor(
                    out=p4[:, :, KP:OUT_K, :], in0=u16_4[:, :, KP:OUT_K, :],
                    in1=fn_b.to_broadcast([128, 2, OUT_K - KP, IV]),
                    op=mybir.AluOpType.mult)

                # ---- fold + reduce once per pair -> tp_pair [128, (t,k)=64]
                tp_pair = tps.tile([128, 64], F16, tag="tp2")
                fold = mid.tile([128, 1472], F16, tag="fold")
                f4 = fold[:].rearrange("p (t k h) -> p t k h", t=2, k=OUT_K)
                nc.vector.tensor_tensor(out=f4, in0=p4[:, :, :, 0:23],
                                        in1=p4[:, :, :, 23:IV], op=mybir.AluOpType.add)
                nc.vector.tensor_reduce(
                    out=tp_pair[:], in_=fold[:].rearrange("p (g h) -> p g h", h=23),
                    axis=mybir.AxisListType.X, op=mybir.AluOpType.add)

                # ---- tail per pair: transpose + 2-way tiled FC
                tpT_ps = psfc.tile([64, 128], F16, tag="tpT_ps")
                nc.tensor.transpose(tpT_ps[:], tp_pair[:], ident[:])
                tpT = tps.tile([64, 128], F16, tag="tpT")
                nc.scalar.copy(out=tpT[:], in_=tpT_ps[:])
                oT_ps = psfc.tile([128, 128], F32, tag="o")
                nc.tensor.matmul(oT_ps[0:REP, 0:128], lhsT=mfc[0:32, :],
                                 rhs=tpT[0:32, :], start=True, stop=True,
                                 tile_position=(0, 0))
                nc.tensor.matmul(oT_ps[64:64 + REP, 0:128], lhsT=mfc[32:64, :],
                                 rhs=tpT[32:64, :], start=True, stop=True,
                                 tile_position=(32, 64))
                oT = io.tile([128, 128], F16, tag="oT")
                nc.scalar.copy(out=oT[0:64 + REP, :], in_=oT_ps[0:64 + REP, :])
                nc.sync.dma_start(out=out_d[:, tp2, :], in_=oT[:])

        lp.__exit__(None, None, None)
    return nc


def _ensure_ntff_hook():
    """Register the axon NTFF profiling hook if the image's antenv lacks
    axon_hooks (boot degrades silently in that case). Enables
    run_bass_kernel_spmd(trace=True) to return exec_time_ns."""
    import contextlib
    import ctypes
    import sys
    import types

    try:
        from antenv.axon_hooks import get_axon_ntff_profile_hook  # noqa: F401
        return
    except ImportError:
        pass
    import antenv

    so_path = "/opt/axon/libaxon_pjrt.so"
    mod = types.ModuleType("antenv.axon_hooks")
    _state = {"hook": None}
    mod.set_axon_ntff_profile_hook = lambda h: _state.__setitem__("hook", h)
    mod.get_axon_ntff_profile_hook = lambda: _state["hook"]
    sys.modules["antenv.axon_hooks"] = mod
    antenv.axon_hooks = mod

    try:
        lib = ctypes.CDLL(so_path)
    except OSError:
        return
    if not hasattr(lib, "axon_start_nrt_profile"):
        return
    lib.axon_start_nrt_profile.argtypes = [ctypes.POINTER(ctypes.c_int64), ctypes.c_size_t]
    lib.axon_start_nrt_profile.restype = ctypes.c_int64
    lib.axon_stop_nrt_profile.argtypes = [ctypes.c_char_p]
    lib.axon_stop_nrt_profile.restype = ctypes.c_int64

    @contextlib.contextmanager
    def _hook(output_dir, device_ids):
        import jax

        jax.devices()
        if device_ids:
            ids = (ctypes.c_int64 * len(device_ids))(*device_ids)
            rc = lib.axon_start_nrt_profile(ids, len(device_ids))
        else:
            rc = lib.axon_start_nrt_profile(None, 0)
        if rc != 0:
            raise RuntimeError(f"axon_start_nrt_profile rc={rc}")
        try:
            yield
        finally:
            n = lib.axon_stop_nrt_profile(str(output_dir).encode())
            print(f"ntff profile: {n} file(s) written to {output_dir}")

    mod.set_axon_ntff_profile_hook(_hook)


_NC_CACHE = None


def _get_nc():
    global _NC_CACHE
    if _NC_CACHE is None:
        _NC_CACHE = _build_bass()
        _split_excess_waits(_NC_CACHE)   # HW-compile legalization
    return _NC_CACHE


def _host_prep(f_nodes, f_edges, edge_index, W_tp, W_fc1, W_fc2):
    sel = _voigt_sel()
    # voigt-selected features, fp16
    fn_v = f_nodes.reshape(-1, 2, RAW)[:, :, sel].astype(np.float16)   # [N, 2, 45]
    fe_v = f_edges.reshape(-1, 2, RAW)[:, :, sel].astype(np.float16)   # [E, 2, 45]
    row = np.asarray(edge_index[0], dtype=np.int64)

    # W2[j, (k, i46)] = W_tp[i, j, k] / 45, duplicated at rows 64-109
    w2 = np.zeros((110, KJ), dtype=np.float16)
    wt = (np.transpose(W_tp.astype(np.float64), (1, 2, 0)) / 45.0)     # [j, k, i]
    w2[0:REP, :] = np.pad(wt, ((0, 0), (0, 0), (0, 1))).reshape(REP, KJ).astype(np.float16)
    w2[64:64 + REP, :] = w2[0:REP, :]

    # Mfc at partition offsets 0 and 32 (2-way tiled FC)
    mfc_np = ((W_fc1.astype(np.float64) @ W_fc2.astype(np.float64))
              / math.sqrt(32.0 * 64.0)).astype(np.float16)             # [32, 45]
    mfc = np.zeros((128, REP), dtype=np.float16)
    mfc[0:32] = mfc_np
    mfc[32:64] = mfc_np

    ident = np.eye(128, dtype=np.float16)
    return fn_v, fe_v, row, w2, mfc, ident


def kernel(f_nodes, f_edges, edge_index, W_tp, W_fc1, W_fc2, _trace=False):
    f_nodes = np.asarray(f_nodes, dtype=np.float32)
    f_edges = np.asarray(f_edges, dtype=np.float32)
    edge_index = np.asarray(edge_index)
    fn_v, fe_v, row, w2, mfc, ident = _host_prep(
        f_nodes, f_edges, edge_index,
        np.asarray(W_tp, np.float32), np.asarray(W_fc1, np.float32),
        np.asarray(W_fc2, np.float32))

    in_maps = []
    for core in range(N_CORES):
        lo = core * E_PER_CORE
        hi = lo + E_PER_CORE
        # FT [110, E_PAD]: voigt(fe)^T, ch0 rows 0-45, ch1 rows 64-109
        ft = np.zeros((128, E_PAD), dtype=np.float16)
        ft[0:REP, :E_PER_CORE] = fe_v[lo:hi, 0, :].T
        ft[64:64 + REP, :E_PER_CORE] = fe_v[lo:hi, 1, :].T
        # FN [T*128, 48]: gathered node rows, (c,e)-packed per 64-edge tile
        r = np.zeros((E_PAD,), dtype=np.int64)
        r[:E_PER_CORE] = row[lo:hi]
        g = fn_v[r]                                    # [E_PAD, 2, 45]
        fn_p = np.zeros((N_TILES, 2, TILE_E, 48), dtype=np.float16)
        fn_p[:, :, :, 0:REP] = g.reshape(N_TILES, TILE_E, 2, REP).transpose(0, 2, 1, 3)
        # [T, (c,e)=128, 48] -> pair-packed [128, T/2, 96]
        fn_pp = fn_p.reshape(N_TILES // 2, 2, 128, 48).transpose(2, 0, 1, 3).reshape(128, N_TILES // 2, 96)
        packed = np.zeros((128, N_TILES // 2, 224), dtype=np.float16)
        packed[:, :, 0:128] = ft.reshape(128, N_TILES // 2, 128)
        packed[:, :, 128:224] = fn_pp
        in_maps.append({
            "inp": packed,
            "w2": w2,
            "mfc": mfc,
            "ident": ident,
        })

    nc = _get_nc()
    if _trace:
        _ensure_ntff_hook()
        import concourse.bass_utils as _BU
        _BU.upload_artifacts = lambda tmpdir: "local://" + str(tmpdir)
    res = run_bass_kernel_spmd(nc, in_maps, list(range(N_CORES)), trace=_trace)

    outs = []
    for core in range(N_CORES):
        o = np.asarray(res.results[core]["out_shard"])   # [128, T/2, 128] fp16
        # rows 0-44: ch0 of even tile? -> decode: pair p: cols 0-127 = (c,e) of
        # tile 2p (rows 0-44 out ch? ...) layout: oT rows 0:45 = tile-a (t even),
        # rows 64:109 = tile-b (t odd); cols = (c, e) 2x64
        o = o.astype(np.float32)
        oa = o[0:REP, :, :].reshape(REP, N_TILES // 2, 2, TILE_E)      # tile 2p
        ob = o[64:64 + REP, :, :].reshape(REP, N_TILES // 2, 2, TILE_E)  # tile 2p+1
        full = np.empty((REP, N_TILES, 2, TILE_E), dtype=np.float32)
        full[:, 0::2] = oa
        full[:, 1::2] = ob
        # -> [E_PAD, 2, 45]
        full = full.transpose(1, 3, 2, 0).reshape(E_PAD, 2, REP)
        outs.append(full[:E_PER_CORE])
    result = np.concatenate(outs, axis=0)
    if _trace:
        return result, res
    return result


# revision 20
# speedup vs baseline: 1.2222x; 1.2222x over previous
"""
Trainium2 Bass kernel for nn_NodeEquiModel (gnn_message_passing).

Computation (reference, jax):
    fn = equi_rep(f_nodes)            # [N, 2, 45]  (45-of-81 selection per 9x9 block)
    fe = equi_rep(f_edges)            # [E, 2, 45]
    fn = fn[edge_index[0]]            # gather -> [E, 2, 45]
    tp[e,c,k] = sum_ij fn[e,c,i] fe[e,c,j] W_tp[i,j,k] / 45
    out = (tp @ W_fc1)/sqrt(32) @ W_fc2 / sqrt(64)    # [E, 2, 45]

Device strategy (8 cores, edges sharded, 50048 edges/core):
  64-edge tiles with channels packed into partitions: rows 0-63 = ch0,
  rows 64-127 = ch1 of the same 64 edges.  Host precomputes (all fp16):
    FT  [110, T*64]  voigt(fe)^T        (ch0 rows 0-45, ch1 rows 64-109)
    FN  [T*128, 48]  voigt(f_nodes)[row] gathered rows, (c,e)-packed
    W2  [110, 1472]  W'[j,(k,i46)] = W_tp[i,j,k]/45 at rows 0-45 and 64-109
    Mfc [ 4*32, 45]  (W_fc1@W_fc2)/sqrt(32*64) replicated at partitions 0,32,64,96
  Per tile:
    PE   pass-1 (2-way array tiling, ch0/ch1 concurrent):
         u[0:64,(k,i)]  = FT_ch0^T @ W2   (3 chunks <= 512)
         u[64:128,...]  = FT_ch1^T @ W2   (quadrant (64,64))
    ACT  evac u -> u16 fp16 (k < KA); DVE fused-mult the rest from PSUM
    DVE/Pool  prod = u16 * fn (fp16 @2x; Pool takes k < KP)
    DVE  fold i-halves (pair-merged), tensor_reduce over 23 -> tp fp16
    PE   (per tile pair) transpose tp2 -> tpT; FC 2-way tiled -> oT fp32
    ACT  evacs; DMA out fp16 [45+45 rows, 128].
  Host: inverse layout -> [E, 2, 45] fp32.
"""

import math

import numpy as np

import concourse.bass as bass
import concourse.mybir as mybir
import concourse.tile as tile
from concourse.bass_utils import run_bass_kernel_spmd

# ---------------------------------------------------------------- constants
N_NODES = 100000
N_EDGES = 400000
MB = 9
RAW = MB * MB          # 81
REP = 45
IV = 46                # padded i dim (45 + 1)
OUT_K = 32
KJ = OUT_K * IV        # 1472
N_CORES = 8

TILE_E = 64            # edges per tile (x2 channels = 128 partitions)
E_PER_CORE = N_EDGES // N_CORES            # 50000
N_TILES = math.ceil(E_PER_CORE / TILE_E)   # 782
E_PAD = N_TILES * TILE_E                   # 50048

KA = 32                # k < KA evac'd by ACT (all of U)
KP = 18                # k < KP: fp16 mult on Pool; KP<=k: fp16 mult on DVE
F16 = mybir.dt.float16
F32 = mybir.dt.float32


def _voigt_sel():
    """45 flat indices into the 81-element 9x9 block, in generate_equi_rep order."""
    idx = [0]
    idx += [9 * i + i for i in range(1, 4)]
    iu, ju = np.triu_indices(3, 1)
    idx += [9 * (i + 1) + (j + 1) for i, j in zip(iu, ju)]
    idx += [9 * i + i for i in range(4, 9)]
    iu, ju = np.triu_indices(5, 1)
    idx += [9 * (i + 4) + (j + 4) for i, j in zip(iu, ju)]
    idx += [j for j in range(1, 4)]
    idx += [j for j in range(4, 9)]
    idx += [9 * i + j for i in range(1, 4) for j in range(4, 9)]
    assert len(idx) == 45 and len(set(idx)) == 45
    return np.array(idx, dtype=np.int64)


def _split_excess_waits(nc):
    """PE matmuls and DMA pseudo-instructions can carry at most ONE sync wait
    on TRN2 (walrus codegen: 'Too many sync wait commands'). Move excess waits
    onto a standalone NoOp on the same engine stream right before the
    instruction."""
    import bass_rust

    f = nc.m.functions[0]
    for b in f.blocks:
        il = b.instructions
        k = 0
        while k < len(il):
            inst = il[k]
            si = inst.sync_info
            if si is not None and len(si.on_wait) > 1:
                moved = list(si.on_wait[:-1])
                kept = [si.on_wait[-1]]
                for w in moved:
                    nop = bass_rust.InstNoOp(name=f"I-wsplit-{nc.next_id()}", ins=[], outs=[])
                    nop.engine = inst.engine
                    nop.sync_info = bass_rust.SyncInfo(on_wait=[w], on_update=[])
                    il.insert(k, nop)
                    k += 1
                inst.sync_info = bass_rust.SyncInfo(on_wait=kept,
                                                    on_update=list(si.on_update))
            k += 1


def _build_bass():
    nc = bass.Bass()

    in_d = nc.declare_dram_parameter("inp", [128, N_TILES // 2, 224], F16, isOutput=False)
    w2_d = nc.declare_dram_parameter("w2", [110, KJ], F16, isOutput=False)
    mfc_d = nc.declare_dram_parameter("mfc", [128, REP], F16, isOutput=False)
    ident_d = nc.declare_dram_parameter("ident", [128, 128], F16, isOutput=False)
    out_d = nc.declare_dram_parameter("out_shard", [128, N_TILES // 2, 128], F16, isOutput=True)

    NK_CHUNKS = [(0, 512), (512, 1024), (1024, KJ)]
    A = KA * IV            # 920 ACT-evac'd columns
    lp = None

    with tile.TileContext(nc) as tc:
        with (
            tc.tile_pool(name="consts", bufs=1) as consts,
            tc.tile_pool(name="io", bufs=4) as io,
            tc.tile_pool(name="mid", bufs=3) as mid,
            tc.tile_pool(name="tps", bufs=3) as tps,
            tc.tile_pool(name="psu", bufs=2, space="PSUM") as psu,
            tc.tile_pool(name="psfc", bufs=1, space="PSUM") as psfc,
        ):
            w2 = consts.tile([110, KJ], F16, tag="w2")
            nc.sync.dma_start(out=w2[:], in_=w2_d[:])
            mfc = consts.tile([128, REP], F16, tag="mfc")
            nc.sync.dma_start(out=mfc[:], in_=mfc_d[:])
            ident = consts.tile([128, 128], F16, tag="id")
            nc.sync.dma_start(out=ident[:], in_=ident_d[:])

            # Preamble warm-up: absorb const-DMA deps into each engine's clock
            # before the loop (PE matmuls carry only one HW sync wait).
            warm = psu.tile([128, KJ], F32, tag="u")
            nc.tensor.matmul(warm[0:64, 0:128], lhsT=w2[0:46, 0:64],
                             rhs=w2[0:46, 0:128], start=True, stop=True,
                             tile_position=(0, 0))
            nc.tensor.matmul(warm[64:128, 0:128], lhsT=w2[64:110, 0:64],
                             rhs=w2[64:110, 0:128], start=True, stop=True,
                             tile_position=(64, 64))
            warmT = psfc.tile([64, 128], F16, tag="tpT_ps")
            nc.tensor.transpose(warmT[:], ident[:, 0:64], ident[:])
            warm2 = psfc.tile([128, 128], F32, tag="o")
            nc.tensor.matmul(warm2[0:45, 0:64], lhsT=mfc[0:32, 0:45],
                             rhs=ident[0:32, 0:64], start=True, stop=True,
                             tile_position=(0, 0))
            nc.tensor.matmul(warm2[64:109, 64:128], lhsT=mfc[32:64, 0:45],
                             rhs=ident[32:64, 0:64], start=True, stop=True,
                             tile_position=(32, 64))

            lp = nc.allow_low_precision("fp16 pipeline; fp32 accumulation on PE")
            lp.__enter__()

            for tp2 in range(N_TILES // 2):
                # ---- pair inputs: one packed DMA
                inp = io.tile([128, 224], F16, tag="inp")
                nc.sync.dma_start(out=inp[:], in_=in_d[:, tp2, :])
                ft = inp[:, 0:128]
                fnv = inp[:, 128:224]

                u16 = mid.tile([128, 2 * KJ], F16, tag="u16")
                for half in range(2):
                    # ---- pass-1: 2-way PE array tiling, channels concurrent
                    u_ps = psu.tile([128, KJ], F32, tag="u")
                    e0 = half * TILE_E
                    for (a, b) in NK_CHUNKS:
                        nc.tensor.matmul(u_ps[0:64, a:b],
                                         lhsT=ft[0:46, e0:e0 + TILE_E],
                                         rhs=w2[0:46, a:b], start=True, stop=True,
                                         tile_position=(0, 0))
                        nc.tensor.matmul(u_ps[64:128, a:b],
                                         lhsT=ft[64:110, e0:e0 + TILE_E],
                                         rhs=w2[64:110, a:b], start=True, stop=True,
                                         tile_position=(64, 64))
                    # ---- ACT evac into pair buffer half
                    nc.scalar.copy(out=u16[:, half * KJ:(half + 1) * KJ], in_=u_ps[:])

                # ---- multiplies -> prod fp16 [128, (2t, k32, i46)], one op per engine
                prod = mid.tile([128, 2 * KJ], F16, tag="prod")
                p4 = prod[:].rearrange("p (t k i) -> p t k i", t=2, k=OUT_K)
                u16_4 = u16[:].rearrange("p (t k i) -> p t k i", t=2, k=OUT_K)
                fn_b = fnv.rearrange("p (t a i) -> p t a i", t=2, a=1)[:, :, :, 0:IV]
                # Pool: k < KP
                nc.gpsimd.tensor_tensor(
                    out=p4[:, :, 0:KP, :], in0=u16_4[:, :, 0:KP, :],
                    in1=fn_b.to_broadcast([128, 2, KP, IV]),
                    op=mybir.AluOpType.mult)
                # DVE: KP <= k
                nc.vector.tensor_tensor(
                    out=p4[:, :, KP:OUT_K, :], in0=u16_4[:, :, KP:OUT_K, :],
                    in1=fn_b.to_broadcast([128, 2, OUT_K - KP, IV]),
                    op=mybir.AluOpType.mult)

                # ---- fold + reduce once per pair -> tp_pair [128, (t,k)=64]
                tp_pair = tps.tile([128, 64], F16, tag="tp2")
                fold = mid.tile([128, 1472], F16, tag="fold")
                f4 = fold[:].rearrange("p (t k h) -> p t k h", t=2, k=OUT_K)
                nc.vector.tensor_tensor(out=f4, in0=p4[:, :, :, 0:23],
                                        in1=p4[:, :, :, 23:IV], op=mybir.AluOpType.add)
                nc.vector.tensor_reduce(
                    out=tp_pair[:], in_=fold[:].rearrange("p (g h) -> p g h", h=23),
                    axis=mybir.AxisListType.X, op=mybir.AluOpType.add)

                # ---- tail per pair: transpose + 2-way tiled FC
                tpT_ps = psfc.tile([64, 128], F16, tag="tpT_ps")
                nc.tensor.transpose(tpT_ps[:], tp_pair[:], ident[:])
                tpT = tps.tile([64, 128], F16, tag="tpT")
                nc.scalar.copy(out=tpT[:], in_=tpT_ps[:])
                oT_ps = psfc.tile([128, 128], F32, tag="o")
                nc.tensor.matmul(oT_ps[0:REP, 0:128], lhsT=mfc[0:32, :],
                                 rhs=tpT[0:32, :], start=True, stop=True,
                                 tile_position=(0, 0))
                nc.tensor.matmul(oT_ps[64:64 + REP, 0:128], lhsT=mfc[32:64, :],
                                 rhs=tpT[32:64, :], start=True, stop=True,
                                 tile_position=(32, 64))
                oT = io.tile([128, 128], F16, tag="oT")
                nc.scalar.copy(out=oT[0:64 + REP, :], in_=oT_ps[0:64 + REP, :])
                nc.sync.dma_start(out=out_d[:, tp2, :], in_=oT[:])

        lp.__exit__(None, None, None)
    return nc


def _ensure_ntff_hook():
    """Register the axon NTFF profiling hook if the image's antenv lacks
    axon_hooks (boot degrades silently in that case). Enables
    run_bass_kernel_spmd(trace=True) to return exec_time_ns."""
    import contextlib
    import ctypes
    import sys
    import types

    try:
        from antenv.axon_hooks import get_axon_ntff_profile_hook  # noqa: F401
        return
    except ImportError:
        pass
    import antenv

    so_path = "/opt/axon/libaxon_pjrt.so"
    mod = types.ModuleType("antenv.axon_hooks")
    _state = {"hook": None}
    mod.set_axon_ntff_profile_hook = lambda h: _state.__setitem__("hook", h)
    mod.get_axon_ntff_profile_hook = lambda: _state["hook"]
    sys.modules["antenv.axon_hooks"] = mod
    antenv.axon_hooks = mod

    try:
        lib = ctypes.CDLL(so_path)
    except OSError:
        return
    if not hasattr(lib, "axon_start_nrt_profile"):
        return
    lib.axon_start_nrt_profile.argtypes = [ctypes.POINTER(ctypes.c_int64), ctypes.c_size_t]
    lib.axon_start_nrt_profile.restype = ctypes.c_int64
    lib.axon_stop_nrt_profile.argtypes = [ctypes.c_char_p]
    lib.axon_stop_nrt_profile.restype = ctypes.c_int64

    @contextlib.contextmanager
    def _hook(output_dir, device_ids):
        import jax

        jax.devices()
        if device_ids:
            ids = (ctypes.c_int64 * len(device_ids))(*device_ids)
            rc = lib.axon_start_nrt_profile(ids, len(device_ids))
        else:
            rc = lib.axon_start_nrt_profile(None, 0)
        if rc != 0:
            raise RuntimeError(f"axon_start_nrt_profile rc={rc}")
        try:
            yield
        finally:
            n = lib.axon_stop_nrt_profile(str(output_dir).encode())
            print(f"ntff profile: {n} file(s) written to {output_dir}")

    mod.set_axon_ntff_profile_hook(_hook)


_NC_CACHE = None


def _get_nc():
    global _NC_CACHE
    if _NC_CACHE is None:
        _NC_CACHE = _build_bass()
        _split_excess_waits(_NC_CACHE)   # HW-compile legalization
    return _NC_CACHE


def _host_prep(f_nodes, f_edges, edge_index, W_tp, W_fc1, W_fc2):
    sel = _voigt_sel()
    # voigt-selected features, fp16
    fn_v = f_nodes.reshape(-1, 2, RAW)[:, :, sel].astype(np.float16)   # [N, 2, 45]
    fe_v = f_edges.reshape(-1, 2, RAW)[:, :, sel].astype(np.float16)   # [E, 2, 45]
    row = np.asarray(edge_index[0], dtype=np.int64)

    # W2[j, (k, i46)] = W_tp[i, j, k] / 45, duplicated at rows 64-109
    w2 = np.zeros((110, KJ), dtype=np.float16)
    wt = (np.transpose(W_tp.astype(np.float64), (1, 2, 0)) / 45.0)     # [j, k, i]
    w2[0:REP, :] = np.pad(wt, ((0, 0), (0, 0), (0, 1))).reshape(REP, KJ).astype(np.float16)
    w2[64:64 + REP, :] = w2[0:REP, :]

    # Mfc at partition offsets 0 and 32 (2-way tiled FC)
    mfc_np = ((W_fc1.astype(np.float64) @ W_fc2.astype(np.float64))
              / math.sqrt(32.0 * 64.0)).astype(np.float16)             # [32, 45]
    mfc = np.zeros((128, REP), dtype=np.float16)
    mfc[0:32] = mfc_np
    mfc[32:64] = mfc_np

    ident = np.eye(128, dtype=np.float16)
    return fn_v, fe_v, row, w2, mfc, ident


def kernel(f_nodes, f_edges, edge_index, W_tp, W_fc1, W_fc2, _trace=False):
    f_nodes = np.asarray(f_nodes, dtype=np.float32)
    f_edges = np.asarray(f_edges, dtype=np.float32)
    edge_index = np.asarray(edge_index)
    fn_v, fe_v, row, w2, mfc, ident = _host_prep(
        f_nodes, f_edges, edge_index,
        np.asarray(W_tp, np.float32), np.asarray(W_fc1, np.float32),
        np.asarray(W_fc2, np.float32))

    in_maps = []
    for core in range(N_CORES):
        lo = core * E_PER_CORE
        hi = lo + E_PER_CORE
        # FT [110, E_PAD]: voigt(fe)^T, ch0 rows 0-45, ch1 rows 64-109
        ft = np.zeros((128, E_PAD), dtype=np.float16)
        ft[0:REP, :E_PER_CORE] = fe_v[lo:hi, 0, :].T
        ft[64:64 + REP, :E_PER_CORE] = fe_v[lo:hi, 1, :].T
        # FN [T*128, 48]: gathered node rows, (c,e)-packed per 64-edge tile
        r = np.zeros((E_PAD,), dtype=np.int64)
        r[:E_PER_CORE] = row[lo:hi]
        g = fn_v[r]                                    # [E_PAD, 2, 45]
        fn_p = np.zeros((N_TILES, 2, TILE_E, 48), dtype=np.float16)
        fn_p[:, :, :, 0:REP] = g.reshape(N_TILES, TILE_E, 2, REP).transpose(0, 2, 1, 3)
        # [T, (c,e)=128, 48] -> pair-packed [128, T/2, 96]
        fn_pp = fn_p.reshape(N_TILES // 2, 2, 128, 48).transpose(2, 0, 1, 3).reshape(128, N_TILES // 2, 96)
        packed = np.zeros((128, N_TILES // 2, 224), dtype=np.float16)
        packed[:, :, 0:128] = ft.reshape(128, N_TILES // 2, 128)
        packed[:, :, 128:224] = fn_pp
        in_maps.append({
            "inp": packed,
            "w2": w2,
            "mfc": mfc,
            "ident": ident,
        })

    nc = _get_nc()
    if _trace:
        _ensure_ntff_hook()
        import concourse.bass_utils as _BU
        _BU.upload_artifacts = lambda tmpdir: "local://" + str(tmpdir)
    res = run_bass_kernel_spmd(nc, in_maps, list(range(N_CORES)), trace=_trace)

    outs = []
    for core in range(N_CORES):
        o = np.asarray(res.results[core]["out_shard"])   # [128, T/2, 128] fp16
        # rows 0-44: ch0 of even tile? -> decode: pair p: cols 0-127 = (c,e) of
        # tile 2p (rows 0-44 out ch? ...) layout: oT rows 0:45 = tile-a (t even),
        # rows 64:109 = tile-b (t odd); cols = (c, e) 2x64
        o = o.astype(np.float32)
        oa = o[0:REP, :, :].reshape(REP, N_TILES // 2, 2, TILE_E)      # tile 2p
        ob = o[64:64 + REP, :, :].reshape(REP, N_TILES // 2, 2, TILE_E)  # tile 2p+1
        full = np.empty((REP, N_TILES, 2, TILE_E), dtype=np.float32)
        full[:, 0::2] = oa
        full[:, 1::2] = ob
        # -> [E_PAD, 2, 45]
        full = full.transpose(1, 3, 2, 0).reshape(E_PAD, 2, REP)
        outs.append(full[:E_PER_CORE])
    result = np.concatenate(outs, axis=0)
    if _trace:
        return result, res
    return result
